# revision 22
# baseline (speedup 1.0000x reference)
"""Trainium2 Bass kernel for nn_Attention2D -- 2D Winograd F(2x2, 3x3), v2.

Reference computation (per batch element b):
    g_em   = img_fvec @ W1.T + b1                       # [HID]
    x_em   = conv3x3_same(patch_fmap, conv_w) + conv_b  # [HID, H, W]
    actv   = tanh(x_em + g_em[:, None, None])           # [HID, H, W]
    logits = W2 @ actv.reshape(HID, HW)                 # [1, HW]
    wts    = softmax(logits)                            # [1, HW]
    attn   = patch_fmap.reshape(C, HW) @ wts.T          # [C]

2D Winograd F(2x2,3x3) cuts PE multiply planes 1.5x vs the 1D variant
(256 vs 384 matmuls of N=392 per image pair).  Division of labour:

    host:   U = G w G^T;  V1[jx] = x-direction B^T combos of the padded
            image, rows padded to 16 cols so every on-device read is
            4B-aligned (DVE 2x mode).  The raw patch never ships: the
            finale reconstructs it from V1[1] +- V1[2].
    DVE:    y-direction combos (stride-2 row slices of V1) -> V[iy,jx],
            stage-A output transform Ty = A^T_y M (PSUM reads), half of
            stage-B, finale weighted sums.
    GPSIMD: the other slices of the y-combos and stage-B (SBUF bf16).
    ACT:    M1 PSUM evictions, batched tanh, softmax exp.
    PE:     per (m, jx): 4 iy x 4 kc accumulating matmuls, both pair
            images in one N=392 moving operand; logits; weight bcast.

The previous pair's epilogue (stage-B, tanh, logits, softmax, weighted
sum) is emitted between this pair's PE groups, so the PE only drains at
the very end.
"""

import numpy as np
import ml_dtypes

import concourse.bass as bass
import concourse.bacc as bacc
import concourse.tile as tile
from concourse import mybir
from concourse.bass_utils import run_bass_kernel_spmd

B = 64
C_IN = 512
HID = 512
H = W = 28
HW = H * W
N_CORES = 8
B_PER_CORE = B // N_CORES      # 8
NPAIRS = B_PER_CORE // 2       # 4
KC = C_IN // 128               # 4
MC = HID // 128                # 4
T = H // 2                     # 14 winograd tiles per dim
VR = H + 2                     # 30 v1 rows
VC = 16                        # v1 row pitch (14 used, padded for align)
NP2 = 2 * T * T                # 392 = matmul N (both images of the pair)
NHALF = HW // 2                # 392 (logit halves)

FP32 = mybir.dt.float32
BF16 = mybir.dt.bfloat16
ADD = mybir.AluOpType.add
SUB = mybir.AluOpType.subtract
MUL = mybir.AluOpType.mult

# F(2,3) B^T combos (same for x and y): d0-d2, d1+d2, d2-d1, d1-d3
BT_COMBOS = [(0, 2, SUB), (1, 2, ADD), (2, 1, SUB), (1, 3, SUB)]


def build_bass():
    nc = bacc.Bacc(None)

    # V2[pp, jx, iy]: fully host-transformed winograd input tiles
    v2_d = nc.dram_tensor("v2", [NPAIRS, 4, 4, KC, 128, 2, T, T], BF16,
                          kind="ExternalInput")
    # V1[b, jx in (1,2)]: x-transformed rows for the finale's raw-patch
    # reconstruction only
    v1_d = nc.dram_tensor("v1", [B_PER_CORE, 2, KC, 128, VR, VC], BF16,
                          kind="ExternalInput")
    u_d = nc.dram_tensor("u2", [4, MC, 4, 128, KC, 128], BF16,
                         kind="ExternalInput")
    imgT_d = nc.dram_tensor("imgT", [C_IN, B_PER_CORE], BF16,
                            kind="ExternalInput")
    w1t_d = nc.dram_tensor("w1t", [C_IN, HID], BF16, kind="ExternalInput")
    w2_d = nc.dram_tensor("w2", [HID], BF16, kind="ExternalInput")
    bsum_d = nc.dram_tensor("bsum", [HID], FP32, kind="ExternalInput")
    out_d = nc.dram_tensor("out", [128, KC, B_PER_CORE], FP32,
                           kind="ExternalOutput")
    ssum_d = nc.dram_tensor("ssum", [1, B_PER_CORE], FP32,
                            kind="ExternalOutput")

    with tile.TileContext(nc) as tc:
        with (
            tc.tile_pool(name="wpool", bufs=1) as wpool,
            tc.tile_pool(name="vkpool", bufs=2) as vkpool,
            tc.tile_pool(name="v2pool", bufs=2) as v2pool,
            tc.tile_pool(name="typool", bufs=2) as typool,
            tc.tile_pool(name="zqpool", bufs=1) as zqpool,
            tc.tile_pool(name="gupool", bufs=1) as gupool,
            tc.tile_pool(name="actvpool", bufs=7) as actvpool,
            tc.tile_pool(name="s1pool", bufs=2) as s1pool,
            tc.tile_pool(name="spool", bufs=1) as spool,
            tc.tile_pool(name="scrpool", bufs=1) as scrpool,
            tc.tile_pool(name="cpool", bufs=5, space="PSUM") as cpool,
            tc.tile_pool(name="lpool", bufs=1, space="PSUM") as lpool,
            tc.tile_pool(name="bpool", bufs=1, space="PSUM") as bpool,
        ):
            w1t_sb = wpool.tile([128, KC, HID], BF16)
            imgT_sb = wpool.tile([128, KC, B_PER_CORE], BF16)
            w2_sb = wpool.tile([128, MC], BF16)
            bsum_sb = wpool.tile([128, MC], FP32)
            u_sb = wpool.tile([128, 4, MC, 4, KC, 128], BF16)
            ones_sb = wpool.tile([1, 128], BF16)
            gbias_sb = wpool.tile([128, MC, B_PER_CORE], FP32)
            attn_sb = wpool.tile([128, KC, B_PER_CORE], FP32)
            ssum_sb = wpool.tile([1, B_PER_CORE], FP32)

            def emit_small_dmas():
                nc.sync.dma_start(
                    out=w1t_sb,
                    in_=w1t_d[:].rearrange("(k p) c -> p k c", p=128))
                nc.sync.dma_start(
                    out=imgT_sb,
                    in_=imgT_d[:].rearrange("(k p) b -> p k b", p=128))
                nc.sync.dma_start(
                    out=w2_sb, in_=w2_d[:].rearrange("(k p) -> p k", p=128))
                nc.sync.dma_start(
                    out=bsum_sb, in_=bsum_d[:].rearrange("(k p) -> p k", p=128))
                nc.gpsimd.memset(ones_sb, 1.0)

            def emit_gbias():
                for m in range(MC):
                    gps = cpool.tile([128, B_PER_CORE], FP32, tag="cps")
                    for k in range(KC):
                        nc.tensor.matmul(
                            gps,
                            w1t_sb[:, k, m * 128:(m + 1) * 128],
                            imgT_sb[:, k, :],
                            start=(k == 0),
                            stop=(k == KC - 1),
                        )
                    nc.scalar.activation(
                        out=gbias_sb[:, m, :],
                        in_=gps,
                        func=mybir.ActivationFunctionType.Identity,
                        bias=bsum_sb[:, m:m + 1],
                        scale=1.0,
                    )

            state = {b: {"actv": {}} for b in range(B_PER_CORE)}  # per image
            pstate = {}   # per pair
            vstate = {}   # (pp, jx) -> v1 tile [128, 2, KC, VR, VC]

            def emit_v1load(pp):
                """jx 1/2 x-transformed rows feed the finale's raw-patch
                reconstruction; they live until the pair's finale2."""
                v1t = vkpool.tile([128, 2, 2, KC, VR, VC], BF16,
                                  tag="v1keep", name=f"v1_{pp}")
                for il in (0, 1):
                    for j in (0, 1):
                        nc.sync.dma_start(
                            out=v1t[:, il, j],
                            in_=v1_d[2 * pp + il, j].rearrange(
                                "k p y c -> p k y c"))
                vstate[pp] = v1t

            def emit_v2load(pp, jx):
                v2 = v2pool.tile([128, 4, KC, 2, T, T], BF16, tag="v2")
                for iy in range(4):
                    nc.sync.dma_start(
                        out=v2[:, iy],
                        in_=v2_d[pp, jx, iy].rearrange(
                            "k p l a c -> p k l a c"))
                return v2

            def emit_conv_j(pp, jx, v2, hook):
                if jx == 0:
                    pstate[pp] = {"ty": {}}
                for m in range(MC):
                    cps_l = []
                    for iy in range(4):
                        cps = cpool.tile([128, NP2], FP32, tag="cps",
                                         name=f"cps{iy}")
                        for k in range(KC):
                            nc.tensor.matmul(
                                cps,
                                u_sb[:, jx, m, iy, k, :],
                                v2[:, iy, k],
                                start=(k == 0),
                                stop=(k == KC - 1),
                            )
                        cps_l.append(cps)
                    if jx == 0:
                        ty = typool.tile([128, 2, 4, NP2], BF16, tag=f"ty{m}",
                                         name=f"ty{m}")
                        pstate[pp]["ty"][m] = ty
                    ty = pstate[pp]["ty"][m]
                    # stage-A: Ty0 = M0+M1+M2, Ty1 = M1-M2-M3.  M1/M2 are
                    # ACT-evicted so only 2 of the 4 DVE ops read PSUM
                    # (PSUM operands force the DVE into 1x mode).
                    s1 = s1pool.tile([128, NP2], BF16, tag="s1")
                    nc.scalar.copy(out=s1, in_=cps_l[1])
                    s2 = s1pool.tile([128, NP2], BF16, tag="s2")
                    nc.scalar.copy(out=s2, in_=cps_l[2])
                    t01 = s1pool.tile([128, NP2], BF16, tag="tt", name="t01")
                    nc.vector.tensor_tensor(out=t01, in0=s1, in1=cps_l[0],
                                            op=ADD)
                    nc.vector.tensor_tensor(out=ty[:, 0, jx], in0=t01,
                                            in1=s2, op=ADD)
                    t12 = s1pool.tile([128, NP2], BF16, tag="tt", name="t12")
                    nc.vector.tensor_tensor(out=t12, in0=s1, in1=s2,
                                            op=SUB)
                    nc.vector.tensor_tensor(out=ty[:, 1, jx], in0=t12,
                                            in1=cps_l[3], op=SUB)
                    hook(m)

            def emit_epilogue_m(pp, m):
                """stage-B x-combines (DVE r=0 / GPSIMD r=1) + tanh."""
                ty = pstate[pp]["ty"][m]
                zq = zqpool.tile([128, 2, 2, NP2], BF16, tag="zq")
                for r in (0, 1):
                    eng = nc.gpsimd
                    u0 = gupool.tile([128, NP2], BF16, tag=f"gu0{r}",
                                     name=f"gu0{r}")
                    eng.tensor_tensor(out=u0, in0=ty[:, r, 0],
                                      in1=ty[:, r, 1], op=ADD)
                    eng.tensor_tensor(out=zq[:, r, 0], in0=u0,
                                      in1=ty[:, r, 2], op=ADD)
                    u1 = gupool.tile([128, NP2], BF16, tag=f"gu1{r}",
                                     name=f"gu1{r}")
                    eng.tensor_tensor(out=u1, in0=ty[:, r, 1],
                                      in1=ty[:, r, 2], op=SUB)
                    eng.tensor_tensor(out=zq[:, r, 1], in0=u1,
                                      in1=ty[:, r, 3], op=SUB)
                for il in (0, 1):
                    b = 2 * pp + il
                    actv_m = actvpool.tile([128, H, W], BF16, tag="actv")
                    state[b]["actv"][m] = actv_m
                    for r in (0, 1):
                        for xp in (0, 1):
                            nc.scalar.activation(
                                out=actv_m[:, r::2, xp::2],
                                in_=zq[:, r, xp, il * T * T:(il + 1) * T * T]
                                .rearrange("p (a c) -> p a c", a=T),
                                func=mybir.ActivationFunctionType.Tanh,
                                bias=gbias_sb[:, m, b:b + 1],
                                scale=1.0,
                            )

            def emit_logits_img(b):
                st = state[b]
                st["lps"] = []
                for h in (0, 1):
                    lps = lpool.tile([1, NHALF], FP32, tag=f"lps{h}",
                                     name=f"lps{h}")
                    st["lps"].append(lps)
                    for m in range(MC):
                        flat = st["actv"][m].rearrange("p a b -> p (a b)")
                        nc.tensor.matmul(
                            lps,
                            w2_sb[:, m:m + 1],
                            flat[:, h * NHALF:(h + 1) * NHALF],
                            start=(m == 0),
                            stop=(m == MC - 1),
                        )

            def emit_finale1a(b):
                l_sb = spool.tile([1, 2, NHALF], FP32, tag=f"l_sb{b % 2}",
                                  name="l_sb")
                for h in (0, 1):
                    nc.scalar.copy(out=l_sb[:, h], in_=state[b]["lps"][h])
                negmax = spool.tile([1, 1], FP32, tag=f"negmax{b % 2}",
                                    name="negmax")
                nc.vector.reduce_max(out=negmax, in_=l_sb,
                                     axis=mybir.AxisListType.XY, negate=True)
                state[b]["l_sb"] = l_sb
                state[b]["negmax"] = negmax

            def emit_finale1b(b):
                l_sb = state[b]["l_sb"]
                negmax = state[b]["negmax"]
                e_sb = spool.tile([1, HW], BF16, tag=f"e_sb{b % 2}",
                                  name="e_sb")
                nc.scalar.activation(
                    out=e_sb.rearrange("p (h n) -> p h n", h=2),
                    in_=l_sb,
                    func=mybir.ActivationFunctionType.Exp,
                    bias=negmax,
                    scale=1.0,
                )
                nc.vector.reduce_sum(out=ssum_sb[:, b:b + 1], in_=e_sb,
                                     axis=mybir.AxisListType.X)
                state[b]["en"] = e_sb

            def emit_finale2(b):
                """weighted raw-feature sum, raw patch reconstructed from
                v1[1] = d1+d2 and v1[2] = d2-d1:
                  2*attn = sum v1[1]*(e_odd+e_even) + v1[2]*(e_odd-e_even)
                (the 1/2 is folded into the host-side ssum divide)."""
                st = state.pop(b)
                pp, il = divmod(b, 2)
                en_rc = st["en"].rearrange("p (r c) -> p r c", r=H)
                e12 = scrpool.tile([1, 2, H, T], BF16, tag="e12")
                nc.vector.tensor_tensor(out=e12[:, 0], in0=en_rc[:, :, 1::2],
                                        in1=en_rc[:, :, 0::2], op=ADD)
                nc.vector.tensor_tensor(out=e12[:, 1], in0=en_rc[:, :, 1::2],
                                        in1=en_rc[:, :, 0::2], op=SUB)
                ebs = scrpool.tile([128, 2, H, T], BF16, tag="ebs")
                for q in range(2):
                    bps = bpool.tile([128, NP2], FP32, tag="bps")
                    nc.tensor.matmul(bps, ones_sb, e12[:, q],
                                     start=True, stop=True)
                    nc.scalar.copy(out=ebs[:, q], in_=bps)
                v1t = vstate[pp]
                for k in range(KC):
                    scr = scrpool.tile([128, 2, H, T], BF16, tag="scr")
                    for q in range(2):
                        nc.vector.tensor_tensor(
                            out=scr[:, q],
                            in0=v1t[:, il, q, k, 1:H + 1, 0:T],
                            in1=ebs[:, q], op=MUL,
                        )
                    nc.scalar.activation(
                        out=scr,
                        in_=scr,
                        func=mybir.ActivationFunctionType.Identity,
                        accum_out=attn_sb[:, k, b:b + 1],
                    )
                    if b == B_PER_CORE - 1:
                        nc.sync.dma_start(out=out_d[:, k], in_=attn_sb[:, k])

            # ---- preamble: critical-path DMA order ----
            emit_small_dmas()
            v2tiles = {}
            # iy-granular, iy-major first loads so the very first PSUM
            # group only waits for ~1 MB of DMA
            v2t0 = v2pool.tile([128, 4, KC, 2, T, T], BF16, tag="v2",
                               name="v2t0")
            for iy in range(4):
                nc.sync.dma_start(
                    out=v2t0[:, iy],
                    in_=v2_d[0, 0, iy].rearrange("k p l a c -> p k l a c"))
                for m in range(MC):
                    nc.sync.dma_start(out=u_sb[:, 0, m, iy], in_=u_d[0, m, iy])
            v2tiles[0] = v2t0
            v2tiles[1] = emit_v2load(0, 1)
            emit_gbias()
            for jx in range(1, 4):
                for m in range(MC):
                    nc.sync.dma_start(
                        out=u_sb[:, jx, m],
                        in_=u_d[jx, m].rearrange("i p k c -> p i k c"))
            emit_v1load(0)

            # ---- pair pipeline ----
            positions = [(pp, jx) for pp in range(NPAIRS) for jx in range(4)]
            for idx, (pp, jx) in enumerate(positions):
                if idx + 2 < len(positions):
                    v2tiles[idx + 2] = emit_v2load(*positions[idx + 2])

                def hook(m, pp=pp, jx=jx):
                    if jx == 0 and m == 3 and pp + 1 < NPAIRS:
                        emit_v1load(pp + 1)
                    if jx == 3 and pp == NPAIRS - 1 and m > 0:
                        # last pair: start its epilogue under its own conv
                        emit_epilogue_m(pp, m - 1)
                    if pp == 0:
                        return
                    q = pp - 1
                    if jx == 1:
                        # exp ops adjacent at m==3 -> fewer ACT
                        # activation-table reloads
                        (emit_logits_img(2 * q) if m == 0 else
                         emit_finale1a(2 * q) if m == 1 else
                         emit_logits_img(2 * q + 1) if m == 2 else
                         (emit_finale1a(2 * q + 1),
                          emit_finale1b(2 * q),
                          emit_finale1b(2 * q + 1)))
                    elif jx == 2 and m < 2:
                        emit_finale2(2 * q + m)
                emit_conv_j(pp, jx, v2tiles.pop(idx), hook)
                if jx == 0 and pp > 0:
                    # batched so the ACT does all 32 tanh back-to-back
                    # (activation-table loads are ~1.3us per func switch)
                    for m in range(MC):
                        emit_epilogue_m(pp - 1, m)

            # tail: last pair's remaining epilogue + finales
            q = NPAIRS - 1
            emit_epilogue_m(q, MC - 1)
            emit_logits_img(2 * q)
            emit_finale1a(2 * q)
            emit_logits_img(2 * q + 1)
            emit_finale1a(2 * q + 1)
            emit_finale1b(2 * q)
            emit_finale1b(2 * q + 1)
            emit_finale2(2 * q)
            emit_finale2(2 * q + 1)

            nc.sync.dma_start(out=ssum_d[:], in_=ssum_sb)

    nc.compile()
    return nc


_CACHED = {}


def get_bass():
    if "nc" not in _CACHED:
        _CACHED["nc"] = build_bass()
    return _CACHED["nc"]


G_MAT = np.array([[1, 0, 0], [0.5, 0.5, 0.5], [0.5, -0.5, 0.5], [0, 0, 1]],
                 np.float32)


def make_in_maps(img_fvec, patch_fmap, W1, b1, conv_w, conv_b, W2, b2):
    img_fvec = np.asarray(img_fvec, dtype=np.float32)
    patch_fmap = np.asarray(patch_fmap, dtype=np.float32)
    W1 = np.asarray(W1, dtype=np.float32)
    b1 = np.asarray(b1, dtype=np.float32)
    conv_w = np.asarray(conv_w, dtype=np.float32)
    conv_b = np.asarray(conv_b, dtype=np.float32)
    W2 = np.asarray(W2, dtype=np.float32)

    w1t = np.ascontiguousarray(W1.T).astype(ml_dtypes.bfloat16)
    w2 = np.ascontiguousarray(W2[0]).astype(ml_dtypes.bfloat16)
    bsum = np.ascontiguousarray(b1 + conv_b).astype(np.float32)

    # U[iy,jx] = G w G^T -> [4jx, MC, 4iy, 128p(cin), KC, 128(cout)]
    u2 = np.einsum("iy,ocyx,jx->ijco", G_MAT, conv_w, G_MAT)
    u2 = u2.reshape(4, 4, KC, 128, MC, 128).transpose(1, 4, 0, 3, 2, 5)
    u2 = np.ascontiguousarray(u2).astype(ml_dtypes.bfloat16)

    # host x-direction B^T combos on the bf16-padded image
    xpad = np.zeros((B, C_IN, H + 2, W + 2), np.float32)
    xpad[:, :, 1:H + 1, 1:W + 1] = patch_fmap.astype(
        ml_dtypes.bfloat16).astype(np.float32)
    d = [xpad[:, :, :, a:a + 2 * T - 1:2] for a in range(4)]  # x taps
    v1 = np.zeros((B, 4, C_IN, VR, VC), np.float32)
    for jx, (a, bb, op) in enumerate(
            [(0, 2, 1), (1, 2, 0), (2, 1, 1), (1, 3, 1)]):
        v1[:, jx, :, :, 0:T] = d[a] - d[bb] if op else d[a] + d[bb]
    # bf16-round v1 (that's what the device DVE produced), then the
    # y-direction combos -> full V2, bf16
    v1b = v1.astype(ml_dtypes.bfloat16).astype(np.float32)
    v2 = np.zeros((B, 4, 4, C_IN, T, T), np.float32)
    for iy, (ya, yb, op) in enumerate(
            [(0, 2, 1), (1, 2, 0), (2, 1, 1), (1, 3, 1)]):
        da = v1b[:, :, :, ya:ya + 2 * T - 1:2, 0:T]
        db = v1b[:, :, :, yb:yb + 2 * T - 1:2, 0:T]
        v2[:, :, iy] = da - db if op else da + db
    # -> [pp, jx, iy, KC, 128, il, T, T]
    v2 = v2.reshape(B // 2, 2, 4, 4, KC, 128, T, T).transpose(
        0, 2, 3, 4, 5, 1, 6, 7)
    v2 = np.ascontiguousarray(v2).astype(ml_dtypes.bfloat16)
    # finale keeps only jx 1, 2 of v1
    v1k = np.ascontiguousarray(
        v1[:, 1:3].reshape(B, 2, KC, 128, VR, VC)).astype(ml_dtypes.bfloat16)

    in_maps = []
    for c in range(N_CORES):
        sl = slice(c * B_PER_CORE, (c + 1) * B_PER_CORE)
        slp = slice(c * NPAIRS, (c + 1) * NPAIRS)
        imgT = np.ascontiguousarray(img_fvec[sl].T).astype(ml_dtypes.bfloat16)
        in_maps.append({
            "v2": v2[slp],
            "v1": v1k[sl],
            "u2": u2,
            "imgT": imgT,
            "w1t": w1t,
            "w2": w2,
            "bsum": bsum,
        })
    return in_maps


def kernel(img_fvec, patch_fmap, W1, b1, conv_w, conv_b, W2, b2,
           trace=False, **run_kwargs):
    nc = get_bass()
    in_maps = make_in_maps(img_fvec, patch_fmap, W1, b1, conv_w, conv_b,
                           W2, b2)
    res = run_bass_kernel_spmd(nc, in_maps, core_ids=list(range(N_CORES)),
                               trace=trace, **run_kwargs)
    # finale computes 2*attn*ssum_scale -> divide by 2*ssum on the host
    out = np.concatenate(
        [(r["out"] / (2.0 * r["ssum"][0][None, None, :]))
         .transpose(2, 1, 0).reshape(B_PER_CORE, C_IN)
         for r in res.results], axis=0)
    if trace:
        kernel.last_results = res
    return out


# revision 23
# speedup vs baseline: 1.0893x; 1.0893x over previous
"""Trainium2 Bass kernel for nn_Attention2D -- 2D Winograd F(2x2, 3x3), v2.

Reference computation (per batch element b):
    g_em   = img_fvec @ W1.T + b1                       # [HID]
    x_em   = conv3x3_same(patch_fmap, conv_w) + conv_b  # [HID, H, W]
    actv   = tanh(x_em + g_em[:, None, None])           # [HID, H, W]
    logits = W2 @ actv.reshape(HID, HW)                 # [1, HW]
    wts    = softmax(logits)                            # [1, HW]
    attn   = patch_fmap.reshape(C, HW) @ wts.T          # [C]

2D Winograd F(2x2,3x3) cuts PE multiply planes 1.5x vs the 1D variant
(256 vs 384 matmuls of N=392 per image pair).  Division of labour:

    host:   U = G w G^T;  V1[jx] = x-direction B^T combos of the padded
            image, rows padded to 16 cols so every on-device read is
            4B-aligned (DVE 2x mode).  The raw patch never ships: the
            finale reconstructs it from V1[1] +- V1[2].
    DVE:    y-direction combos (stride-2 row slices of V1) -> V[iy,jx],
            stage-A output transform Ty = A^T_y M (PSUM reads), half of
            stage-B, finale weighted sums.
    GPSIMD: the other slices of the y-combos and stage-B (SBUF bf16).
    ACT:    M1 PSUM evictions, batched tanh, softmax exp.
    PE:     per (m, jx): 4 iy x 4 kc accumulating matmuls, both pair
            images in one N=392 moving operand; logits; weight bcast.

The previous pair's epilogue (stage-B, tanh, logits, softmax, weighted
sum) is emitted between this pair's PE groups, so the PE only drains at
the very end.
"""

import numpy as np
import ml_dtypes

import concourse.bass as bass
import concourse.bacc as bacc
import concourse.tile as tile
from concourse import mybir
from concourse.bass_utils import run_bass_kernel_spmd

B = 64
C_IN = 512
HID = 512
H = W = 28
HW = H * W
N_CORES = 8
B_PER_CORE = B // N_CORES      # 8
NPAIRS = B_PER_CORE // 2       # 4
KC = C_IN // 128               # 4
MC = HID // 128                # 4
T = H // 2                     # 14 winograd tiles per dim
VR = H + 2                     # 30 v1 rows
VC = 16                        # v1 row pitch (14 used, padded for align)
NP2 = 2 * T * T                # 392 = matmul N (both images of the pair)
NHALF = HW // 2                # 392 (logit halves)

FP32 = mybir.dt.float32
BF16 = mybir.dt.bfloat16
ADD = mybir.AluOpType.add
SUB = mybir.AluOpType.subtract
MUL = mybir.AluOpType.mult

# F(2,3) B^T combos (same for x and y): d0-d2, d1+d2, d2-d1, d1-d3
BT_COMBOS = [(0, 2, SUB), (1, 2, ADD), (2, 1, SUB), (1, 3, SUB)]


def build_bass():
    nc = bacc.Bacc(None)

    # V2[pp, jx, iy]: fully host-transformed winograd input tiles
    v2_d = nc.dram_tensor("v2", [NPAIRS, 4, 4, KC, 128, 2, T, T], BF16,
                          kind="ExternalInput")
    # V1[b, jx in (1,2)]: x-transformed rows for the finale's raw-patch
    # reconstruction only
    v1_d = nc.dram_tensor("v1", [B_PER_CORE, 2, KC, 128, VR, VC], BF16,
                          kind="ExternalInput")
    u_d = nc.dram_tensor("u2", [4, MC, 4, 128, KC, 128], BF16,
                         kind="ExternalInput")
    imgT_d = nc.dram_tensor("imgT", [C_IN, B_PER_CORE], BF16,
                            kind="ExternalInput")
    w1t_d = nc.dram_tensor("w1t", [C_IN, HID], BF16, kind="ExternalInput")
    w2_d = nc.dram_tensor("w2", [HID], BF16, kind="ExternalInput")
    bsum_d = nc.dram_tensor("bsum", [HID], FP32, kind="ExternalInput")
    out_d = nc.dram_tensor("out", [128, KC, B_PER_CORE], FP32,
                           kind="ExternalOutput")
    ssum_d = nc.dram_tensor("ssum", [1, B_PER_CORE], FP32,
                            kind="ExternalOutput")

    with tile.TileContext(nc) as tc:
        with (
            tc.tile_pool(name="wpool", bufs=1) as wpool,
            tc.tile_pool(name="vkpool", bufs=2) as vkpool,
            tc.tile_pool(name="v2pool", bufs=2) as v2pool,
            tc.tile_pool(name="typool", bufs=2) as typool,
            tc.tile_pool(name="zqpool", bufs=1) as zqpool,
            tc.tile_pool(name="gupool", bufs=1) as gupool,
            tc.tile_pool(name="actvpool", bufs=7) as actvpool,
            tc.tile_pool(name="s1pool", bufs=2) as s1pool,
            tc.tile_pool(name="spool", bufs=1) as spool,
            tc.tile_pool(name="scrpool", bufs=1) as scrpool,
            tc.tile_pool(name="cpool", bufs=5, space="PSUM") as cpool,
            tc.tile_pool(name="lpool", bufs=1, space="PSUM") as lpool,
            tc.tile_pool(name="bpool", bufs=1, space="PSUM") as bpool,
        ):
            w1t_sb = wpool.tile([128, KC, HID], BF16)
            imgT_sb = wpool.tile([128, KC, B_PER_CORE], BF16)
            w2_sb = wpool.tile([128, MC], BF16)
            bsum_sb = wpool.tile([128, MC], FP32)
            u_sb = wpool.tile([128, 4, MC, 4, KC, 128], BF16)
            ones_sb = wpool.tile([1, 128], BF16)
            gbias_sb = wpool.tile([128, MC, B_PER_CORE], FP32)
            attn_sb = wpool.tile([128, KC, B_PER_CORE], FP32)
            ssum_sb = wpool.tile([1, B_PER_CORE], FP32)

            def emit_small_dmas():
                nc.sync.dma_start(
                    out=w1t_sb,
                    in_=w1t_d[:].rearrange("(k p) c -> p k c", p=128))
                nc.sync.dma_start(
                    out=imgT_sb,
                    in_=imgT_d[:].rearrange("(k p) b -> p k b", p=128))
                nc.sync.dma_start(
                    out=w2_sb, in_=w2_d[:].rearrange("(k p) -> p k", p=128))
                nc.sync.dma_start(
                    out=bsum_sb, in_=bsum_d[:].rearrange("(k p) -> p k", p=128))
                nc.gpsimd.memset(ones_sb, 1.0)

            def emit_gbias():
                for m in range(MC):
                    gps = cpool.tile([128, B_PER_CORE], FP32, tag="cps")
                    for k in range(KC):
                        nc.tensor.matmul(
                            gps,
                            w1t_sb[:, k, m * 128:(m + 1) * 128],
                            imgT_sb[:, k, :],
                            start=(k == 0),
                            stop=(k == KC - 1),
                        )
                    nc.scalar.activation(
                        out=gbias_sb[:, m, :],
                        in_=gps,
                        func=mybir.ActivationFunctionType.Identity,
                        bias=bsum_sb[:, m:m + 1],
                        scale=1.0,
                    )

            state = {b: {"actv": {}} for b in range(B_PER_CORE)}  # per image
            pstate = {}   # per pair
            vstate = {}   # (pp, jx) -> v1 tile [128, 2, KC, VR, VC]

            def emit_v1load(pp):
                """jx 1/2 x-transformed rows feed the finale's raw-patch
                reconstruction; they live until the pair's finale2."""
                v1t = vkpool.tile([128, 2, 2, KC, VR, VC], BF16,
                                  tag="v1keep", name=f"v1_{pp}")
                for il in (0, 1):
                    for j in (0, 1):
                        nc.sync.dma_start(
                            out=v1t[:, il, j],
                            in_=v1_d[2 * pp + il, j].rearrange(
                                "k p y c -> p k y c"))
                vstate[pp] = v1t

            def emit_v2load(pp, jx):
                v2 = v2pool.tile([128, 4, KC, 2, T, T], BF16, tag="v2")
                for iy in range(4):
                    nc.sync.dma_start(
                        out=v2[:, iy],
                        in_=v2_d[pp, jx, iy].rearrange(
                            "k p l a c -> p k l a c"))
                return v2

            def emit_conv_j(pp, jx, v2, hook):
                if jx == 0:
                    pstate[pp] = {"ty": {}}
                for m in range(MC):
                    cps_l = []
                    for iy in range(4):
                        cps = cpool.tile([128, NP2], FP32, tag="cps",
                                         name=f"cps{iy}")
                        for k in range(KC):
                            nc.tensor.matmul(
                                cps,
                                u_sb[:, jx, m, iy, k, :],
                                v2[:, iy, k],
                                start=(k == 0),
                                stop=(k == KC - 1),
                            )
                        cps_l.append(cps)
                    if jx == 0:
                        ty = typool.tile([128, 2, 4, NP2], BF16, tag=f"ty{m}",
                                         name=f"ty{m}")
                        pstate[pp]["ty"][m] = ty
                    ty = pstate[pp]["ty"][m]
                    # stage-A: Ty0 = M0+M1+M2, Ty1 = M1-M2-M3.  M1/M2 are
                    # ACT-evicted so only 2 of the 4 DVE ops read PSUM
                    # (PSUM operands force the DVE into 1x mode).
                    s1 = s1pool.tile([128, NP2], BF16, tag="s1")
                    nc.scalar.copy(out=s1, in_=cps_l[1])
                    s2 = s1pool.tile([128, NP2], BF16, tag="s2")
                    nc.scalar.copy(out=s2, in_=cps_l[2])
                    t01 = s1pool.tile([128, NP2], BF16, tag="tt", name="t01")
                    nc.vector.tensor_tensor(out=t01, in0=s1, in1=cps_l[0],
                                            op=ADD)
                    nc.vector.tensor_tensor(out=ty[:, 0, jx], in0=t01,
                                            in1=s2, op=ADD)
                    t12 = s1pool.tile([128, NP2], BF16, tag="tt", name="t12")
                    nc.vector.tensor_tensor(out=t12, in0=s1, in1=s2,
                                            op=SUB)
                    nc.vector.tensor_tensor(out=ty[:, 1, jx], in0=t12,
                                            in1=cps_l[3], op=SUB)
                    hook(m)

            def emit_epilogue_m(pp, m):
                """stage-B x-combines (DVE r=0 / GPSIMD r=1) + tanh."""
                ty = pstate[pp]["ty"][m]
                zq = zqpool.tile([128, 2, 2, NP2], BF16, tag="zq")
                for r in (0, 1):
                    eng = nc.vector if r == 0 else nc.gpsimd
                    u0 = gupool.tile([128, NP2], BF16, tag=f"gu0{r}",
                                     name=f"gu0{r}")
                    eng.tensor_tensor(out=u0, in0=ty[:, r, 0],
                                      in1=ty[:, r, 1], op=ADD)
                    eng.tensor_tensor(out=zq[:, r, 0], in0=u0,
                                      in1=ty[:, r, 2], op=ADD)
                    u1 = gupool.tile([128, NP2], BF16, tag=f"gu1{r}",
                                     name=f"gu1{r}")
                    eng.tensor_tensor(out=u1, in0=ty[:, r, 1],
                                      in1=ty[:, r, 2], op=SUB)
                    eng.tensor_tensor(out=zq[:, r, 1], in0=u1,
                                      in1=ty[:, r, 3], op=SUB)
                for il in (0, 1):
                    b = 2 * pp + il
                    actv_m = actvpool.tile([128, H, W], BF16, tag="actv")
                    state[b]["actv"][m] = actv_m
                    for r in (0, 1):
                        for xp in (0, 1):
                            nc.scalar.activation(
                                out=actv_m[:, r::2, xp::2],
                                in_=zq[:, r, xp, il * T * T:(il + 1) * T * T]
                                .rearrange("p (a c) -> p a c", a=T),
                                func=mybir.ActivationFunctionType.Tanh,
                                bias=gbias_sb[:, m, b:b + 1],
                                scale=1.0,
                            )

            def emit_logits_img(b):
                st = state[b]
                st["lps"] = []
                for h in (0, 1):
                    lps = lpool.tile([1, NHALF], FP32, tag=f"lps{h}",
                                     name=f"lps{h}")
                    st["lps"].append(lps)
                    for m in range(MC):
                        flat = st["actv"][m].rearrange("p a b -> p (a b)")
                        nc.tensor.matmul(
                            lps,
                            w2_sb[:, m:m + 1],
                            flat[:, h * NHALF:(h + 1) * NHALF],
                            start=(m == 0),
                            stop=(m == MC - 1),
                        )

            def emit_finale1a(b):
                l_sb = spool.tile([1, 2, NHALF], FP32, tag=f"l_sb{b % 2}",
                                  name="l_sb")
                for h in (0, 1):
                    nc.scalar.copy(out=l_sb[:, h], in_=state[b]["lps"][h])
                negmax = spool.tile([1, 1], FP32, tag=f"negmax{b % 2}",
                                    name="negmax")
                nc.vector.reduce_max(out=negmax, in_=l_sb,
                                     axis=mybir.AxisListType.XY, negate=True)
                state[b]["l_sb"] = l_sb
                state[b]["negmax"] = negmax

            def emit_finale1b(b):
                l_sb = state[b]["l_sb"]
                negmax = state[b]["negmax"]
                e_sb = spool.tile([1, HW], BF16, tag=f"e_sb{b % 2}",
                                  name="e_sb")
                nc.scalar.activation(
                    out=e_sb.rearrange("p (h n) -> p h n", h=2),
                    in_=l_sb,
                    func=mybir.ActivationFunctionType.Exp,
                    bias=negmax,
                    scale=1.0,
                )
                nc.vector.reduce_sum(out=ssum_sb[:, b:b + 1], in_=e_sb,
                                     axis=mybir.AxisListType.X)
                state[b]["en"] = e_sb

            def emit_finale2(b):
                """weighted raw-feature sum, raw patch reconstructed from
                v1[1] = d1+d2 and v1[2] = d2-d1:
                  2*attn = sum v1[1]*(e_odd+e_even) + v1[2]*(e_odd-e_even)
                (the 1/2 is folded into the host-side ssum divide)."""
                st = state.pop(b)
                pp, il = divmod(b, 2)
                en_rc = st["en"].rearrange("p (r c) -> p r c", r=H)
                e12 = scrpool.tile([1, 2, H, T], BF16, tag="e12")
                nc.vector.tensor_tensor(out=e12[:, 0], in0=en_rc[:, :, 1::2],
                                        in1=en_rc[:, :, 0::2], op=ADD)
                nc.vector.tensor_tensor(out=e12[:, 1], in0=en_rc[:, :, 1::2],
                                        in1=en_rc[:, :, 0::2], op=SUB)
                ebs = scrpool.tile([128, 2, H, T], BF16, tag="ebs")
                for q in range(2):
                    bps = bpool.tile([128, NP2], FP32, tag="bps")
                    nc.tensor.matmul(bps, ones_sb, e12[:, q],
                                     start=True, stop=True)
                    nc.scalar.copy(out=ebs[:, q], in_=bps)
                v1t = vstate[pp]
                for k in range(KC):
                    scr = scrpool.tile([128, 2, H, T], BF16, tag="scr")
                    for q in range(2):
                        nc.vector.tensor_tensor(
                            out=scr[:, q],
                            in0=v1t[:, il, q, k, 1:H + 1, 0:T],
                            in1=ebs[:, q], op=MUL,
                        )
                    nc.scalar.activation(
                        out=scr,
                        in_=scr,
                        func=mybir.ActivationFunctionType.Identity,
                        accum_out=attn_sb[:, k, b:b + 1],
                    )
                    if b == B_PER_CORE - 1:
                        nc.sync.dma_start(out=out_d[:, k], in_=attn_sb[:, k])

            # ---- preamble: critical-path DMA order ----
            emit_small_dmas()
            v2tiles = {}
            # iy-granular, iy-major first loads so the very first PSUM
            # group only waits for ~1 MB of DMA
            v2t0 = v2pool.tile([128, 4, KC, 2, T, T], BF16, tag="v2",
                               name="v2t0")
            for iy in range(4):
                nc.sync.dma_start(
                    out=v2t0[:, iy],
                    in_=v2_d[0, 0, iy].rearrange("k p l a c -> p k l a c"))
                for m in range(MC):
                    nc.sync.dma_start(out=u_sb[:, 0, m, iy], in_=u_d[0, m, iy])
            v2tiles[0] = v2t0
            v2tiles[1] = emit_v2load(0, 1)
            emit_gbias()
            for jx in range(1, 4):
                for m in range(MC):
                    nc.sync.dma_start(
                        out=u_sb[:, jx, m],
                        in_=u_d[jx, m].rearrange("i p k c -> p i k c"))
            emit_v1load(0)

            # ---- pair pipeline ----
            positions = [(pp, jx) for pp in range(NPAIRS) for jx in range(4)]
            for idx, (pp, jx) in enumerate(positions):
                if idx + 2 < len(positions):
                    v2tiles[idx + 2] = emit_v2load(*positions[idx + 2])

                def hook(m, pp=pp, jx=jx):
                    if jx == 0 and m == 3 and pp + 1 < NPAIRS:
                        emit_v1load(pp + 1)
                    if jx == 3 and pp == NPAIRS - 1 and m > 0:
                        # last pair: start its epilogue under its own conv
                        emit_epilogue_m(pp, m - 1)
                    if pp == 0:
                        return
                    q = pp - 1
                    if jx == 1:
                        # exp ops adjacent at m==3 -> fewer ACT
                        # activation-table reloads
                        (emit_logits_img(2 * q) if m == 0 else
                         emit_finale1a(2 * q) if m == 1 else
                         emit_logits_img(2 * q + 1) if m == 2 else
                         (emit_finale1a(2 * q + 1),
                          emit_finale1b(2 * q),
                          emit_finale1b(2 * q + 1)))
                    elif jx == 2 and m < 2:
                        emit_finale2(2 * q + m)
                emit_conv_j(pp, jx, v2tiles.pop(idx), hook)
                if jx == 0 and pp > 0:
                    # batched so the ACT does all 32 tanh back-to-back
                    # (activation-table loads are ~1.3us per func switch)
                    for m in range(MC):
                        emit_epilogue_m(pp - 1, m)

            # tail: last pair's remaining epilogue + finales
            q = NPAIRS - 1
            emit_epilogue_m(q, MC - 1)
            emit_logits_img(2 * q)
            emit_finale1a(2 * q)
            emit_logits_img(2 * q + 1)
            emit_finale1a(2 * q + 1)
            emit_finale1b(2 * q)
            emit_finale1b(2 * q + 1)
            emit_finale2(2 * q)
            emit_finale2(2 * q + 1)

            nc.sync.dma_start(out=ssum_d[:], in_=ssum_sb)

    nc.compile()
    return nc


_CACHED = {}


def get_bass():
    if "nc" not in _CACHED:
        _CACHED["nc"] = build_bass()
    return _CACHED["nc"]


G_MAT = np.array([[1, 0, 0], [0.5, 0.5, 0.5], [0.5, -0.5, 0.5], [0, 0, 1]],
                 np.float32)


def make_in_maps(img_fvec, patch_fmap, W1, b1, conv_w, conv_b, W2, b2):
    img_fvec = np.asarray(img_fvec, dtype=np.float32)
    patch_fmap = np.asarray(patch_fmap, dtype=np.float32)
    W1 = np.asarray(W1, dtype=np.float32)
    b1 = np.asarray(b1, dtype=np.float32)
    conv_w = np.asarray(conv_w, dtype=np.float32)
    conv_b = np.asarray(conv_b, dtype=np.float32)
    W2 = np.asarray(W2, dtype=np.float32)

    w1t = np.ascontiguousarray(W1.T).astype(ml_dtypes.bfloat16)
    w2 = np.ascontiguousarray(W2[0]).astype(ml_dtypes.bfloat16)
    bsum = np.ascontiguousarray(b1 + conv_b).astype(np.float32)

    # U[iy,jx] = G w G^T -> [4jx, MC, 4iy, 128p(cin), KC, 128(cout)]
    u2 = np.einsum("iy,ocyx,jx->ijco", G_MAT, conv_w, G_MAT)
    u2 = u2.reshape(4, 4, KC, 128, MC, 128).transpose(1, 4, 0, 3, 2, 5)
    u2 = np.ascontiguousarray(u2).astype(ml_dtypes.bfloat16)

    # host x-direction B^T combos on the bf16-padded image
    xpad = np.zeros((B, C_IN, H + 2, W + 2), np.float32)
    xpad[:, :, 1:H + 1, 1:W + 1] = patch_fmap.astype(
        ml_dtypes.bfloat16).astype(np.float32)
    d = [xpad[:, :, :, a:a + 2 * T - 1:2] for a in range(4)]  # x taps
    v1 = np.zeros((B, 4, C_IN, VR, VC), np.float32)
    for jx, (a, bb, op) in enumerate(
            [(0, 2, 1), (1, 2, 0), (2, 1, 1), (1, 3, 1)]):
        v1[:, jx, :, :, 0:T] = d[a] - d[bb] if op else d[a] + d[bb]
    # bf16-round v1 (that's what the device DVE produced), then the
    # y-direction combos -> full V2, bf16
    v1b = v1.astype(ml_dtypes.bfloat16).astype(np.float32)
    v2 = np.zeros((B, 4, 4, C_IN, T, T), np.float32)
    for iy, (ya, yb, op) in enumerate(
            [(0, 2, 1), (1, 2, 0), (2, 1, 1), (1, 3, 1)]):
        da = v1b[:, :, :, ya:ya + 2 * T - 1:2, 0:T]
        db = v1b[:, :, :, yb:yb + 2 * T - 1:2, 0:T]
        v2[:, :, iy] = da - db if op else da + db
    # -> [pp, jx, iy, KC, 128, il, T, T]
    v2 = v2.reshape(B // 2, 2, 4, 4, KC, 128, T, T).transpose(
        0, 2, 3, 4, 5, 1, 6, 7)
    v2 = np.ascontiguousarray(v2).astype(ml_dtypes.bfloat16)
    # finale keeps only jx 1, 2 of v1
    v1k = np.ascontiguousarray(
        v1[:, 1:3].reshape(B, 2, KC, 128, VR, VC)).astype(ml_dtypes.bfloat16)

    in_maps = []
    for c in range(N_CORES):
        sl = slice(c * B_PER_CORE, (c + 1) * B_PER_CORE)
        slp = slice(c * NPAIRS, (c + 1) * NPAIRS)
        imgT = np.ascontiguousarray(img_fvec[sl].T).astype(ml_dtypes.bfloat16)
        in_maps.append({
            "v2": v2[slp],
            "v1": v1k[sl],
            "u2": u2,
            "imgT": imgT,
            "w1t": w1t,
            "w2": w2,
            "bsum": bsum,
        })
    return in_maps


def kernel(img_fvec, patch_fmap, W1, b1, conv_w, conv_b, W2, b2,
           trace=False, **run_kwargs):
    nc = get_bass()
    in_maps = make_in_maps(img_fvec, patch_fmap, W1, b1, conv_w, conv_b,
                           W2, b2)
    res = run_bass_kernel_spmd(nc, in_maps, core_ids=list(range(N_CORES)),
                               trace=trace, **run_kwargs)
    # finale computes 2*attn*ssum_scale -> divide by 2*ssum on the host
    out = np.concatenate(
        [(r["out"] / (2.0 * r["ssum"][0][None, None, :]))
         .transpose(2, 1, 0).reshape(B_PER_CORE, C_IN)
         for r in res.results], axis=0)
    if trace:
        kernel.last_results = res
    return out


# revision 24
# speedup vs baseline: 1.1157x; 1.0243x over previous
"""Trainium2 Bass kernel for nn_Attention2D -- 2D Winograd F(2x2, 3x3), v2.

Reference computation (per batch element b):
    g_em   = img_fvec @ W1.T + b1                       # [HID]
    x_em   = conv3x3_same(patch_fmap, conv_w) + conv_b  # [HID, H, W]
    actv   = tanh(x_em + g_em[:, None, None])           # [HID, H, W]
    logits = W2 @ actv.reshape(HID, HW)                 # [1, HW]
    wts    = softmax(logits)                            # [1, HW]
    attn   = patch_fmap.reshape(C, HW) @ wts.T          # [C]

2D Winograd F(2x2,3x3) cuts PE multiply planes 1.5x vs the 1D variant
(256 vs 384 matmuls of N=392 per image pair).  Division of labour:

    host:   U = G w G^T;  V1[jx] = x-direction B^T combos of the padded
            image, rows padded to 16 cols so every on-device read is
            4B-aligned (DVE 2x mode).  The raw patch never ships: the
            finale reconstructs it from V1[1] +- V1[2].
    DVE:    y-direction combos (stride-2 row slices of V1) -> V[iy,jx],
            stage-A output transform Ty = A^T_y M (PSUM reads), half of
            stage-B, finale weighted sums.
    GPSIMD: the other slices of the y-combos and stage-B (SBUF bf16).
    ACT:    M1 PSUM evictions, batched tanh, softmax exp.
    PE:     per (m, jx): 4 iy x 4 kc accumulating matmuls, both pair
            images in one N=392 moving operand; logits; weight bcast.

The previous pair's epilogue (stage-B, tanh, logits, softmax, weighted
sum) is emitted between this pair's PE groups, so the PE only drains at
the very end.
"""

import numpy as np
import ml_dtypes

import concourse.bass as bass
import concourse.bacc as bacc
import concourse.tile as tile
from concourse import mybir
from concourse.bass_utils import run_bass_kernel_spmd

B = 64
C_IN = 512
HID = 512
H = W = 28
HW = H * W
N_CORES = 8
B_PER_CORE = B // N_CORES      # 8
NPAIRS = B_PER_CORE // 2       # 4
KC = C_IN // 128               # 4
MC = HID // 128                # 4
T = H // 2                     # 14 winograd tiles per dim
VR = H + 2                     # 30 v1 rows
VC = 16                        # v1 row pitch (14 used, padded for align)
NP2 = 2 * T * T                # 392 = matmul N (both images of the pair)
NHALF = HW // 2                # 392 (logit halves)

FP32 = mybir.dt.float32
BF16 = mybir.dt.bfloat16
ADD = mybir.AluOpType.add
SUB = mybir.AluOpType.subtract
MUL = mybir.AluOpType.mult

# F(2,3) B^T combos (same for x and y): d0-d2, d1+d2, d2-d1, d1-d3
BT_COMBOS = [(0, 2, SUB), (1, 2, ADD), (2, 1, SUB), (1, 3, SUB)]


def build_bass():
    nc = bacc.Bacc(None)

    # V2[pp, jx, iy]: fully host-transformed winograd input tiles
    v2_d = nc.dram_tensor("v2", [NPAIRS, 4, 4, KC, 128, 2, T, T], BF16,
                          kind="ExternalInput")
    # V1[b, jx in (1,2)]: x-transformed rows for the finale's raw-patch
    # reconstruction only
    v1_d = nc.dram_tensor("v1", [B_PER_CORE, 2, KC, 128, VR, VC], BF16,
                          kind="ExternalInput")
    u_d = nc.dram_tensor("u2", [4, MC, 4, 128, KC, 128], BF16,
                         kind="ExternalInput")
    imgT_d = nc.dram_tensor("imgT", [C_IN, B_PER_CORE], BF16,
                            kind="ExternalInput")
    w1t_d = nc.dram_tensor("w1t", [C_IN, HID], BF16, kind="ExternalInput")
    w2_d = nc.dram_tensor("w2", [HID], BF16, kind="ExternalInput")
    bsum_d = nc.dram_tensor("bsum", [HID], FP32, kind="ExternalInput")
    out_d = nc.dram_tensor("out", [128, KC, B_PER_CORE], FP32,
                           kind="ExternalOutput")
    ssum_d = nc.dram_tensor("ssum", [1, B_PER_CORE], FP32,
                            kind="ExternalOutput")

    with tile.TileContext(nc) as tc:
        with (
            tc.tile_pool(name="wpool", bufs=1) as wpool,
            tc.tile_pool(name="vkpool", bufs=2) as vkpool,
            tc.tile_pool(name="v2pool", bufs=2) as v2pool,
            tc.tile_pool(name="typool", bufs=2) as typool,
            tc.tile_pool(name="zqpool", bufs=1) as zqpool,
            tc.tile_pool(name="gupool", bufs=1) as gupool,
            tc.tile_pool(name="actvpool", bufs=7) as actvpool,
            tc.tile_pool(name="s1pool", bufs=2) as s1pool,
            tc.tile_pool(name="spool", bufs=1) as spool,
            tc.tile_pool(name="scrpool", bufs=1) as scrpool,
            tc.tile_pool(name="cpool", bufs=5, space="PSUM") as cpool,
            tc.tile_pool(name="lpool", bufs=1, space="PSUM") as lpool,
            tc.tile_pool(name="bpool", bufs=1, space="PSUM") as bpool,
        ):
            w1t_sb = wpool.tile([128, KC, HID], BF16)
            imgT_sb = wpool.tile([128, KC, B_PER_CORE], BF16)
            w2_sb = wpool.tile([128, MC], BF16)
            bsum_sb = wpool.tile([128, MC], FP32)
            u_sb = wpool.tile([128, 4, MC, 4, KC, 128], BF16)
            ones_sb = wpool.tile([1, 128], BF16)
            gbias_sb = wpool.tile([128, MC, B_PER_CORE], FP32)
            attn_sb = wpool.tile([128, KC, B_PER_CORE], FP32)
            ssum_sb = wpool.tile([1, B_PER_CORE], FP32)

            def emit_small_dmas():
                nc.sync.dma_start(
                    out=w1t_sb,
                    in_=w1t_d[:].rearrange("(k p) c -> p k c", p=128))
                nc.sync.dma_start(
                    out=imgT_sb,
                    in_=imgT_d[:].rearrange("(k p) b -> p k b", p=128))
                nc.sync.dma_start(
                    out=w2_sb, in_=w2_d[:].rearrange("(k p) -> p k", p=128))
                nc.sync.dma_start(
                    out=bsum_sb, in_=bsum_d[:].rearrange("(k p) -> p k", p=128))
                nc.gpsimd.memset(ones_sb, 1.0)

            def emit_gbias():
                for m in range(MC):
                    gps = cpool.tile([128, B_PER_CORE], FP32, tag="cps")
                    for k in range(KC):
                        nc.tensor.matmul(
                            gps,
                            w1t_sb[:, k, m * 128:(m + 1) * 128],
                            imgT_sb[:, k, :],
                            start=(k == 0),
                            stop=(k == KC - 1),
                        )
                    nc.scalar.activation(
                        out=gbias_sb[:, m, :],
                        in_=gps,
                        func=mybir.ActivationFunctionType.Identity,
                        bias=bsum_sb[:, m:m + 1],
                        scale=1.0,
                    )

            state = {b: {"actv": {}} for b in range(B_PER_CORE)}  # per image
            pstate = {}   # per pair
            vstate = {}   # (pp, jx) -> v1 tile [128, 2, KC, VR, VC]

            def emit_v1load(pp):
                """jx 1/2 x-transformed rows feed the finale's raw-patch
                reconstruction; they live until the pair's finale2."""
                v1t = vkpool.tile([128, 2, 2, KC, VR, VC], BF16,
                                  tag="v1keep", name=f"v1_{pp}")
                for il in (0, 1):
                    for j in (0, 1):
                        nc.sync.dma_start(
                            out=v1t[:, il, j],
                            in_=v1_d[2 * pp + il, j].rearrange(
                                "k p y c -> p k y c"))
                vstate[pp] = v1t

            def emit_v2load(pp, jx):
                v2 = v2pool.tile([128, 4, KC, 2, T, T], BF16, tag="v2")
                for iy in range(4):
                    nc.sync.dma_start(
                        out=v2[:, iy],
                        in_=v2_d[pp, jx, iy].rearrange(
                            "k p l a c -> p k l a c"))
                return v2

            def emit_conv_j(pp, jx, v2, hook):
                if jx == 0:
                    pstate[pp] = {"ty": {}}
                for m in range(MC):
                    cps_l = []
                    for iy in range(4):
                        cps = cpool.tile([128, NP2], FP32, tag="cps",
                                         name=f"cps{iy}")
                        for k in range(KC):
                            nc.tensor.matmul(
                                cps,
                                u_sb[:, jx, m, iy, k, :],
                                v2[:, iy, k],
                                start=(k == 0),
                                stop=(k == KC - 1),
                            )
                        cps_l.append(cps)
                    if jx == 0:
                        ty = typool.tile([128, 2, 4, NP2], BF16, tag=f"ty{m}",
                                         name=f"ty{m}")
                        pstate[pp]["ty"][m] = ty
                    ty = pstate[pp]["ty"][m]
                    # stage-A: Ty0 = M0+M1+M2, Ty1 = M1-M2-M3.  M1/M2 are
                    # ACT-evicted so only 2 of the 4 DVE ops read PSUM
                    # (PSUM operands force the DVE into 1x mode).
                    s1 = s1pool.tile([128, NP2], BF16, tag="s1")
                    nc.scalar.copy(out=s1, in_=cps_l[1])
                    s2 = s1pool.tile([128, NP2], BF16, tag="s2")
                    nc.scalar.copy(out=s2, in_=cps_l[2])
                    t01 = s1pool.tile([128, NP2], BF16, tag="tt", name="t01")
                    nc.vector.tensor_tensor(out=t01, in0=s1, in1=cps_l[0],
                                            op=ADD)
                    nc.vector.tensor_tensor(out=ty[:, 0, jx], in0=t01,
                                            in1=s2, op=ADD)
                    t12 = s1pool.tile([128, NP2], BF16, tag="tt", name="t12")
                    nc.vector.tensor_tensor(out=t12, in0=s1, in1=s2,
                                            op=SUB)
                    nc.vector.tensor_tensor(out=ty[:, 1, jx], in0=t12,
                                            in1=cps_l[3], op=SUB)
                    hook(m)

            def emit_epilogue_m(pp, m):
                """stage-B x-combines (DVE r=0 / GPSIMD r=1) + tanh."""
                ty = pstate[pp]["ty"][m]
                zq = zqpool.tile([128, 2, 2, NP2], BF16, tag="zq")
                for r in (0, 1):
                    eng = nc.vector if r == 0 else nc.gpsimd
                    u0 = gupool.tile([128, NP2], BF16, tag=f"gu0{r}",
                                     name=f"gu0{r}")
                    eng.tensor_tensor(out=u0, in0=ty[:, r, 0],
                                      in1=ty[:, r, 1], op=ADD)
                    eng.tensor_tensor(out=zq[:, r, 0], in0=u0,
                                      in1=ty[:, r, 2], op=ADD)
                    u1 = gupool.tile([128, NP2], BF16, tag=f"gu1{r}",
                                     name=f"gu1{r}")
                    eng.tensor_tensor(out=u1, in0=ty[:, r, 1],
                                      in1=ty[:, r, 2], op=SUB)
                    eng.tensor_tensor(out=zq[:, r, 1], in0=u1,
                                      in1=ty[:, r, 3], op=SUB)
                for il in (0, 1):
                    b = 2 * pp + il
                    actv_m = actvpool.tile([128, H, W], BF16, tag="actv")
                    state[b]["actv"][m] = actv_m
                    for r in (0, 1):
                        for xp in (0, 1):
                            nc.scalar.activation(
                                out=actv_m[:, r::2, xp::2],
                                in_=zq[:, r, xp, il * T * T:(il + 1) * T * T]
                                .rearrange("p (a c) -> p a c", a=T),
                                func=mybir.ActivationFunctionType.Tanh,
                                bias=gbias_sb[:, m, b:b + 1],
                                scale=1.0,
                            )

            def emit_logits_img(b):
                st = state[b]
                st["lps"] = []
                for h in (0, 1):
                    lps = lpool.tile([1, NHALF], FP32, tag=f"lps{h}",
                                     name=f"lps{h}")
                    st["lps"].append(lps)
                    for m in range(MC):
                        flat = st["actv"][m].rearrange("p a b -> p (a b)")
                        nc.tensor.matmul(
                            lps,
                            w2_sb[:, m:m + 1],
                            flat[:, h * NHALF:(h + 1) * NHALF],
                            start=(m == 0),
                            stop=(m == MC - 1),
                        )

            def emit_finale1a(b):
                l_sb = spool.tile([1, 2, NHALF], FP32, tag=f"l_sb{b % 2}",
                                  name="l_sb")
                for h in (0, 1):
                    nc.scalar.copy(out=l_sb[:, h], in_=state[b]["lps"][h])
                negmax = spool.tile([1, 1], FP32, tag=f"negmax{b % 2}",
                                    name="negmax")
                nc.vector.reduce_max(out=negmax, in_=l_sb,
                                     axis=mybir.AxisListType.XY, negate=True)
                state[b]["l_sb"] = l_sb
                state[b]["negmax"] = negmax

            def emit_finale1b(b):
                l_sb = state[b]["l_sb"]
                negmax = state[b]["negmax"]
                e_sb = spool.tile([1, HW], BF16, tag=f"e_sb{b % 2}",
                                  name="e_sb")
                nc.scalar.activation(
                    out=e_sb.rearrange("p (h n) -> p h n", h=2),
                    in_=l_sb,
                    func=mybir.ActivationFunctionType.Exp,
                    bias=negmax,
                    scale=1.0,
                )
                nc.vector.reduce_sum(out=ssum_sb[:, b:b + 1], in_=e_sb,
                                     axis=mybir.AxisListType.X)
                state[b]["en"] = e_sb

            def emit_finale2(b):
                """weighted raw-feature sum, raw patch reconstructed from
                v1[1] = d1+d2 and v1[2] = d2-d1:
                  2*attn = sum v1[1]*(e_odd+e_even) + v1[2]*(e_odd-e_even)
                (the 1/2 is folded into the host-side ssum divide)."""
                st = state.pop(b)
                pp, il = divmod(b, 2)
                en_rc = st["en"].rearrange("p (r c) -> p r c", r=H)
                e12 = scrpool.tile([1, 2, H, T], BF16, tag="e12")
                nc.vector.tensor_tensor(out=e12[:, 0], in0=en_rc[:, :, 1::2],
                                        in1=en_rc[:, :, 0::2], op=ADD)
                nc.vector.tensor_tensor(out=e12[:, 1], in0=en_rc[:, :, 1::2],
                                        in1=en_rc[:, :, 0::2], op=SUB)
                ebs = scrpool.tile([128, 2, H, T], BF16, tag="ebs")
                for q in range(2):
                    bps = bpool.tile([128, NP2], FP32, tag="bps")
                    nc.tensor.matmul(bps, ones_sb, e12[:, q],
                                     start=True, stop=True)
                    nc.scalar.copy(out=ebs[:, q], in_=bps)
                v1t = vstate[pp]
                for k in range(KC):
                    scr = scrpool.tile([128, 2, H, T], BF16, tag="scr")
                    for q in range(2):
                        nc.vector.tensor_tensor(
                            out=scr[:, q],
                            in0=v1t[:, il, q, k, 1:H + 1, 0:T],
                            in1=ebs[:, q], op=MUL,
                        )
                    nc.scalar.activation(
                        out=scr,
                        in_=scr,
                        func=mybir.ActivationFunctionType.Identity,
                        accum_out=attn_sb[:, k, b:b + 1],
                    )
                    if b == B_PER_CORE - 1:
                        nc.sync.dma_start(out=out_d[:, k], in_=attn_sb[:, k])

            # ---- preamble: critical-path DMA order ----
            emit_small_dmas()
            v2tiles = {}
            v2tiles[0] = emit_v2load(0, 0)
            for m in range(MC):
                nc.sync.dma_start(
                    out=u_sb[:, 0, m],
                    in_=u_d[0, m].rearrange("i p k c -> p i k c"))
            v2tiles[1] = emit_v2load(0, 1)
            emit_gbias()
            for jx in range(1, 4):
                for m in range(MC):
                    nc.sync.dma_start(
                        out=u_sb[:, jx, m],
                        in_=u_d[jx, m].rearrange("i p k c -> p i k c"))
            emit_v1load(0)

            # ---- pair pipeline ----
            positions = [(pp, jx) for pp in range(NPAIRS) for jx in range(4)]
            for idx, (pp, jx) in enumerate(positions):
                if idx + 2 < len(positions):
                    v2tiles[idx + 2] = emit_v2load(*positions[idx + 2])

                def hook(m, pp=pp, jx=jx):
                    if jx == 1 and m == 0 and pp >= 1:
                        # current pair's finale v1 (needed next pair);
                        # its pool slot (pp-2's) is already free, so this
                        # never stalls the in-order sync queue
                        emit_v1load(pp)
                    if jx == 3 and pp == NPAIRS - 1 and m > 0:
                        # last pair: start its epilogue under its own conv
                        emit_epilogue_m(pp, m - 1)
                    if pp == 0:
                        return
                    q = pp - 1
                    # deferred prev-pair work, placed late enough that the
                    # PE never reaches a logits matmul before its tanh
                    # chain (stage-A -> stage-B -> ACT) has finished
                    if jx == 1:
                        if m == 2:
                            emit_logits_img(2 * q)
                        elif m == 3:
                            emit_finale1a(2 * q)
                    elif jx == 2:
                        if m == 0:
                            emit_logits_img(2 * q + 1)
                        elif m == 1:
                            emit_finale1a(2 * q + 1)
                        elif m == 2:
                            emit_finale1b(2 * q)
                            emit_finale1b(2 * q + 1)
                        else:
                            emit_finale2(2 * q)
                    elif jx == 3 and m == 0:
                        emit_finale2(2 * q + 1)
                emit_conv_j(pp, jx, v2tiles.pop(idx), hook)
                if jx == 0 and pp > 0:
                    # batched so the ACT does all 32 tanh back-to-back
                    # (activation-table loads are ~1.3us per func switch)
                    for m in range(MC):
                        emit_epilogue_m(pp - 1, m)

            # tail: last pair's remaining epilogue + finales
            q = NPAIRS - 1
            emit_epilogue_m(q, MC - 1)
            emit_logits_img(2 * q)
            emit_finale1a(2 * q)
            emit_logits_img(2 * q + 1)
            emit_finale1a(2 * q + 1)
            emit_finale1b(2 * q)
            emit_finale1b(2 * q + 1)
            emit_finale2(2 * q)
            emit_finale2(2 * q + 1)

            nc.sync.dma_start(out=ssum_d[:], in_=ssum_sb)

    nc.compile()
    return nc


_CACHED = {}


def get_bass():
    if "nc" not in _CACHED:
        _CACHED["nc"] = build_bass()
    return _CACHED["nc"]


G_MAT = np.array([[1, 0, 0], [0.5, 0.5, 0.5], [0.5, -0.5, 0.5], [0, 0, 1]],
                 np.float32)


def make_in_maps(img_fvec, patch_fmap, W1, b1, conv_w, conv_b, W2, b2):
    img_fvec = np.asarray(img_fvec, dtype=np.float32)
    patch_fmap = np.asarray(patch_fmap, dtype=np.float32)
    W1 = np.asarray(W1, dtype=np.float32)
    b1 = np.asarray(b1, dtype=np.float32)
    conv_w = np.asarray(conv_w, dtype=np.float32)
    conv_b = np.asarray(conv_b, dtype=np.float32)
    W2 = np.asarray(W2, dtype=np.float32)

    w1t = np.ascontiguousarray(W1.T).astype(ml_dtypes.bfloat16)
    w2 = np.ascontiguousarray(W2[0]).astype(ml_dtypes.bfloat16)
    bsum = np.ascontiguousarray(b1 + conv_b).astype(np.float32)

    # U[iy,jx] = G w G^T -> [4jx, MC, 4iy, 128p(cin), KC, 128(cout)]
    u2 = np.einsum("iy,ocyx,jx->ijco", G_MAT, conv_w, G_MAT)
    u2 = u2.reshape(4, 4, KC, 128, MC, 128).transpose(1, 4, 0, 3, 2, 5)
    u2 = np.ascontiguousarray(u2).astype(ml_dtypes.bfloat16)

    # host x-direction B^T combos on the bf16-padded image
    xpad = np.zeros((B, C_IN, H + 2, W + 2), np.float32)
    xpad[:, :, 1:H + 1, 1:W + 1] = patch_fmap.astype(
        ml_dtypes.bfloat16).astype(np.float32)
    d = [xpad[:, :, :, a:a + 2 * T - 1:2] for a in range(4)]  # x taps
    v1 = np.zeros((B, 4, C_IN, VR, VC), np.float32)
    for jx, (a, bb, op) in enumerate(
            [(0, 2, 1), (1, 2, 0), (2, 1, 1), (1, 3, 1)]):
        v1[:, jx, :, :, 0:T] = d[a] - d[bb] if op else d[a] + d[bb]
    # bf16-round v1 (that's what the device DVE produced), then the
    # y-direction combos -> full V2, bf16
    v1b = v1.astype(ml_dtypes.bfloat16).astype(np.float32)
    v2 = np.zeros((B, 4, 4, C_IN, T, T), np.float32)
    for iy, (ya, yb, op) in enumerate(
            [(0, 2, 1), (1, 2, 0), (2, 1, 1), (1, 3, 1)]):
        da = v1b[:, :, :, ya:ya + 2 * T - 1:2, 0:T]
        db = v1b[:, :, :, yb:yb + 2 * T - 1:2, 0:T]
        v2[:, :, iy] = da - db if op else da + db
    # -> [pp, jx, iy, KC, 128, il, T, T]
    v2 = v2.reshape(B // 2, 2, 4, 4, KC, 128, T, T).transpose(
        0, 2, 3, 4, 5, 1, 6, 7)
    v2 = np.ascontiguousarray(v2).astype(ml_dtypes.bfloat16)
    # finale keeps only jx 1, 2 of v1
    v1k = np.ascontiguousarray(
        v1[:, 1:3].reshape(B, 2, KC, 128, VR, VC)).astype(ml_dtypes.bfloat16)

    in_maps = []
    for c in range(N_CORES):
        sl = slice(c * B_PER_CORE, (c + 1) * B_PER_CORE)
        slp = slice(c * NPAIRS, (c + 1) * NPAIRS)
        imgT = np.ascontiguousarray(img_fvec[sl].T).astype(ml_dtypes.bfloat16)
        in_maps.append({
            "v2": v2[slp],
            "v1": v1k[sl],
            "u2": u2,
            "imgT": imgT,
            "w1t": w1t,
            "w2": w2,
            "bsum": bsum,
        })
    return in_maps


def kernel(img_fvec, patch_fmap, W1, b1, conv_w, conv_b, W2, b2,
           trace=False, **run_kwargs):
    nc = get_bass()
    in_maps = make_in_maps(img_fvec, patch_fmap, W1, b1, conv_w, conv_b,
                           W2, b2)
    res = run_bass_kernel_spmd(nc, in_maps, core_ids=list(range(N_CORES)),
                               trace=trace, **run_kwargs)
    # finale computes 2*attn*ssum_scale -> divide by 2*ssum on the host
    out = np.concatenate(
        [(r["out"] / (2.0 * r["ssum"][0][None, None, :]))
         .transpose(2, 1, 0).reshape(B_PER_CORE, C_IN)
         for r in res.results], axis=0)
    if trace:
        kernel.last_results = res
    return out
